# revision 11
# baseline (speedup 1.0000x reference)
"""Trainium2 Bass kernel for nn_Conditional_29222957482793 (retrieval_knn).

reference semantics:
    p = protos[cls_ids]; pl = proto_labels[cls_ids]
    dot = einsum('nd,npd->np', feats, p)
    d_cos = 1 - dot / (max(|f|,1e-8) * max(|p|,1e-8))
    d_l1  = mean|f-p|;  d_l2 = mean (f-p)^2
    probs = mean_t softmax(1/(d_t + 1e-5));  labels = pl[argmax probs]

Sharding: boxes grouped by class (host permutation), class-pure 16-slot tiles
assigned to cores as contiguous spans. Each tile's proto slab is replicated
per-tile in that core's DRAM input so the SPMD instruction stream is
data-independent (all 8 cores run one NEFF; only DRAM contents differ).

Per-core pipeline:
  A) f2/p2 norms: ACT squares -> PE stationary-matmul partition reduces;
     per-class p2 distributed to boxes via a host-provided one-hot matmul
  B) main loop over tiles: DMA slab -> fused DVE |f-p| (custom op, 1x pass)
     -> PE: stationary matmuls vs ones => per-(box,proto) sum_d |f-p|
     -> PE: dots (protoT stationary, fT moving) accumulated over chunks
  C) relayouts via DRAM bounce into pair-major [128, (j, g2, .)] layout
  D) phase-3 math: d_cos/d_l1/d_l2, 3x softmax, mean, argmax, label select
"""

import numpy as np
from contextlib import ExitStack

N, D, C, P = 4096, 512, 100, 64
NCORES = 8
TILE = 16            # slots (boxes) per tile; class-pure
NCHUNK = 4           # D = 4 * 128

_BUILT = {}
LAST_RESULT = None


# --------------------------------------------------------------------------
# host-side planning
# --------------------------------------------------------------------------

def _plan(cls_ids):
    """Class-pure tiles, contiguous spans per core (keeps classes/core low)."""
    order = np.argsort(cls_ids, kind="stable")
    tiles = []
    for c in range(C):
        boxes = order[cls_ids[order] == c]
        for i in range(0, len(boxes), TILE):
            tiles.append((c, boxes[i:i + TILE]))
    ntot = len(tiles)
    base, extra = divmod(ntot, NCORES)
    NT = base + (1 if extra else 0)
    core_tiles, pos = [], 0
    for i in range(NCORES):
        n = base + (1 if i < extra else 0)
        ct = tiles[pos:pos + n]
        pos += n
        while len(ct) < NT:
            ct.append((0, np.array([], dtype=np.int64)))
        core_tiles.append(ct)
    return core_tiles, NT


def _core_inputs(core_t, NT, CLSMAX, feats, protosT, pl_f32):
    S = NT * TILE
    NS3 = (S + 255) // 256
    classes = sorted({c for c, _ in core_t})
    cls_local = {c: i for i, c in enumerate(classes)}
    assert len(classes) <= CLSMAX

    ft = np.zeros((NCHUNK, 128, S), dtype=np.float32)
    ptile = np.zeros((NT, NCHUNK, 128, P), dtype=np.float32)
    pl_pm = np.zeros((NS3 * 2 * 128, P), dtype=np.float32)
    oneh = np.zeros((NS3 * 2, CLSMAX, 128), dtype=np.float32)
    p_comp = np.zeros((NCHUNK, 128, CLSMAX * P), dtype=np.float32)
    slot_box = np.full(S, -1, dtype=np.int64)

    for i, c in enumerate(classes):
        p_comp[:, :, i * P:(i + 1) * P] = protosT[c].reshape(NCHUNK, 128, P)

    for t, (c, boxes) in enumerate(core_t):
        ptile[t] = protosT[c].reshape(NCHUNK, 128, P)
        s0, nb = t * TILE, len(boxes)
        if nb:
            ft[:, :, s0:s0 + nb] = feats[boxes].T.reshape(NCHUNK, 128, nb)
            slot_box[s0:s0 + nb] = boxes
        for k in range(TILE):
            s = s0 + k
            j, g2, part = s // 256, s % 2, (s % 256) // 2
            pl_pm[(j * 2 + g2) * 128 + part] = pl_f32[c]
            oneh[j * 2 + g2, cls_local[c], part] = 1.0
    return dict(ft=ft, ptile=ptile, pl=pl_pm, oneh=oneh, pcomp=p_comp,
                slot_box=slot_box, S=S, NS3=NS3)


# --------------------------------------------------------------------------
# custom DVE op: out = |in0 - in1|
# --------------------------------------------------------------------------

def _register_absdiff():
    import concourse.dve_ops as dve_ops
    from concourse.dve_spec import Spec, Src0, Src1, Zero, maxx, lower
    from concourse.dve_uop import DveOpSpec

    for o in dve_ops.OPS:
        if o.name == "ABSDIFF_ANT":
            return o
    d = Src0 - Src1
    spec = Spec(
        body=maxx(d, Zero - d),
        reference=lambda in0, in1, s0=0.0, s1=0.0, imm2=0.0: np.abs(in0 - in1),
    )
    shas = {}
    for ver in ("v3", "v4"):
        tmp = DveOpSpec(name="ABSDIFF_ANT", opcode=0,
                        uops=lower(spec, ver=ver), rd1_en=True)
        shas[ver] = tmp.sha(ver)
    op = dve_ops.DveOp("ABSDIFF_ANT", spec, subdim=False, uops_sha=shas)
    dve_ops.OPS.append(op)
    dve_ops.CUSTOM_DVE_SPECS[op.name] = op.spec
    dve_ops._SUB_OPCODE_FOR_NAME[op.name] = (
        dve_ops._CUSTOM_DVE_ROW_BASE + len(dve_ops.OPS) - 1)
    return op


# --------------------------------------------------------------------------
# the bass program
# --------------------------------------------------------------------------

def _build(NT, CLSMAX):
    import concourse.bacc as bacc
    import concourse.mybir as mybir
    import concourse.tile as tile

    f32 = mybir.dt.float32
    i32 = mybir.dt.int32
    Alu = mybir.AluOpType
    Act = mybir.ActivationFunctionType
    X = mybir.AxisListType.X
    absdiff = _register_absdiff()

    S = NT * TILE
    NS3 = (S + 255) // 256
    SP = NS3 * 256
    NBLK = NT * 8                 # 2-slot pair blocks
    NFB = (S + 127) // 128
    NPB = (CLSMAX * P + 127) // 128
    NSL = NS3 * 2

    nc = bacc.Bacc("TRN2", target_bir_lowering=False, debug=False,
                   num_devices=NCORES)

    ft_d = nc.dram_tensor("ft", (NCHUNK, 128, S), f32, kind="ExternalInput")
    pt_d = nc.dram_tensor("ptile", (NT, NCHUNK, 128, P), f32, kind="ExternalInput")
    pl_d = nc.dram_tensor("pl", (NSL * 128, P), f32, kind="ExternalInput")
    oh_d = nc.dram_tensor("oneh", (NSL, CLSMAX, 128), f32, kind="ExternalInput")
    pc_d = nc.dram_tensor("pcomp", (NCHUNK, 128, CLSMAX * P), f32, kind="ExternalInput")

    probs_d = nc.dram_tensor("probs_o", (NSL * 128, P), f32, kind="ExternalOutput")
    lab_d = nc.dram_tensor("lab_o", (NSL * 128,), f32, kind="ExternalOutput")

    with tile.TileContext(nc) as tc, ExitStack() as ctx:
        sb = ctx.enter_context(tc.tile_pool(name="sb", bufs=1))
        sb2 = ctx.enter_context(tc.tile_pool(name="sb2", bufs=2))
        sbd = ctx.enter_context(tc.tile_pool(name="sbd", bufs=3))
        ps = ctx.enter_context(tc.tile_pool(name="ps", bufs=1, space="PSUM"))
        psd = ctx.enter_context(tc.tile_pool(name="psd", bufs=2, space="PSUM"))
        dr = ctx.enter_context(tc.tile_pool(name="dr", bufs=1, space="DRAM"))
        dr2 = ctx.enter_context(tc.tile_pool(name="dr2", bufs=2, space="DRAM"))

        F2SZ = max(NFB * 128, SP)
        l1scr = dr.tile([SP, P], f32, tag="l1scr")
        dscr = dr.tile([SP, P], f32, tag="dscr")
        f2scr = dr.tile([F2SZ], f32, tag="f2scr")
        p2scr = dr.tile([NPB * 128], f32, tag="p2scr")

        # ---- persistent SBUF ----
        ft_sb = sb.tile([128, NCHUNK, S], f32, tag="ft")
        nc.sync.dma_start(ft_sb[:], ft_d[:].rearrange("c d s -> d c s"))
        ones_sb = sb.tile([128, 1], f32, tag="ones")
        nc.vector.memset(ones_sb[:], 1.0)
        zer_sb = sb.tile([128, P], f32, tag="zer")
        nc.vector.memset(zer_sb[:], 0.0)

        # zero-fill scratch rows beyond S (garbage boxes read in phase C)
        if SP > S:
            for r0 in range(S, SP, 128):
                nr = min(128, SP - r0)
                nc.sync.dma_start(l1scr[r0:r0 + nr, :], zer_sb[0:nr, :])
                nc.sync.dma_start(dscr[r0:r0 + nr, :], zer_sb[0:nr, :])
        if F2SZ > S:
            for r0 in range(S, F2SZ, 128):
                nr = min(128, F2SZ - r0)
                nc.sync.dma_start(f2scr[r0:r0 + nr], zer_sb[0:nr, 0])

        # ---- phase A: f2 and p2 ----
        sqf = sb.tile([128, NCHUNK, S], f32, tag="sqf")
        nc.scalar.activation(sqf[:], ft_sb[:], Act.Square)
        f2ps = ps.tile([128, NFB, NCHUNK], f32, tag="psA")
        nc.vector.memset(f2ps[:], 0.0)
        for b in range(NFB):
            s0, s1 = b * 128, min((b + 1) * 128, S)
            for c in range(NCHUNK):
                nc.tensor.matmul(f2ps[0:s1 - s0, b, c:c + 1],
                                 sqf[:, c, s0:s1], ones_sb[:],
                                 start=True, stop=True)
        f2sb = sb.tile([128, NFB], f32, tag="f2sb")
        nc.vector.tensor_reduce(out=f2sb[:], in_=f2ps[:], axis=X, op=Alu.add)
        nc.sync.dma_start(
            f2scr[0:NFB * 128].rearrange("(b q) -> q b", q=128), f2sb[:])

        pc_sb = sb.tile([128, NCHUNK, CLSMAX * P], f32, tag="pc")
        nc.sync.dma_start(pc_sb[:], pc_d[:].rearrange("c d q -> d c q"))
        sqp = sb.tile([128, NCHUNK, CLSMAX * P], f32, tag="sqp")
        nc.scalar.activation(sqp[:], pc_sb[:], Act.Square)
        p2ps = ps.tile([128, NPB, NCHUNK], f32, tag="psA")
        nc.vector.memset(p2ps[:], 0.0)
        for b in range(NPB):
            q0, q1 = b * 128, min((b + 1) * 128, CLSMAX * P)
            for c in range(NCHUNK):
                nc.tensor.matmul(p2ps[0:q1 - q0, b, c:c + 1],
                                 sqp[:, c, q0:q1], ones_sb[:],
                                 start=True, stop=True)
        p2sb = sb.tile([128, NPB], f32, tag="p2sb")
        nc.vector.tensor_reduce(out=p2sb[:], in_=p2ps[:], axis=X, op=Alu.add)
        nc.sync.dma_start(
            p2scr[:].rearrange("(b q) -> q b", q=128), p2sb[:])
        p2cls = sb.tile([CLSMAX, P], f32, tag="p2cls")
        nc.sync.dma_start(p2cls[:],
                          p2scr[0:CLSMAX * P].rearrange("(k p) -> k p", p=P))

        # one-hot distribute p2 to boxes (pair-major)
        oh_sb = sb.tile([CLSMAX, NSL, 128], f32, tag="oh")
        nc.sync.dma_start(oh_sb[:], oh_d[:].rearrange("s k q -> k s q"))
        p2pm = sb.tile([128, NSL, P], f32, tag="p2pm")
        for sl in range(NSL):
            op2 = psd.tile([P, 128], f32, tag="pdyn")
            nc.tensor.matmul(op2[:], p2cls[:], oh_sb[:, sl, :],
                             start=True, stop=True)
            o2sb = sb2.tile([P, 128], f32, tag="o2sb")
            nc.scalar.copy(o2sb[:], op2[:])
            p2b = dr2.tile([128, P], f32, tag="p2b")
            nc.sync.dma_start(p2b[:].rearrange("q p -> p q"), o2sb[:])
            nc.sync.dma_start(p2pm[:, sl, :], p2b[:])

        # ---- phase B: main loop ----
        l1ps = ps.tile([128, NBLK, NCHUNK], f32, tag="l1ps")
        dsb = sb.tile([P, S], f32, tag="dsb")
        for t in range(NT):
            pt_sb = sb2.tile([128, NCHUNK, P], f32, tag="pt")
            nc.sync.dma_start(pt_sb[:], pt_d[t].rearrange("c d p -> d c p"))
            absd = sb2.tile([128, NCHUNK, TILE, P], f32, tag="absd")
            for c in range(NCHUNK):
                nc.vector._custom_dve(
                    absdiff,
                    out=absd[:, c, :, :],
                    in0=ft_sb[:, c, t * TILE:(t + 1) * TILE]
                        .unsqueeze(2).broadcast_to([128, TILE, P]),
                    in1=pt_sb[:, c, :].unsqueeze(1).broadcast_to([128, TILE, P]),
                )
            for r in range(8):
                blk = t * 8 + r
                for c in range(NCHUNK):
                    nc.tensor.matmul(l1ps[:, blk, c:c + 1],
                                     absd[:, c, 2 * r:2 * r + 2, :], ones_sb[:],
                                     start=True, stop=True)
            dps = psd.tile([P, TILE], f32, tag="pdyn")
            for c in range(NCHUNK):
                nc.tensor.matmul(dps[:], pt_sb[:, c, :],
                                 ft_sb[:, c, t * TILE:(t + 1) * TILE],
                                 start=(c == 0), stop=(c == NCHUNK - 1))
            nc.scalar.copy(dsb[:, t * TILE:(t + 1) * TILE], dps[:])

        # fold the 4 chunk partials: [128, NBLK, 4] -> [128, NBLK]
        l1cols = sb.tile([128, NBLK], f32, tag="l1cols")
        nc.vector.tensor_reduce(out=l1cols[:], in_=l1ps[:], axis=X, op=Alu.add)

        # ---- phase C: relayouts via DRAM bounce ----
        # l1cols[(g2,p), blk] -> l1scr[box = 2 blk + g2, p]
        for g in range(2):
            nc.sync.dma_start(
                l1scr[g:2 * NBLK:2, :].rearrange("b p -> p b"),
                l1cols[g * P:(g + 1) * P, :])
        nc.sync.dma_start(dscr[0:S, :].rearrange("s p -> p s"), dsb[:])

        l1pm = sb.tile([128, NS3, 2, P], f32, tag="l1pm")
        dotpm = sb.tile([128, NS3, 2, P], f32, tag="dotpm")
        f2pm = sb.tile([128, NS3, 2], f32, tag="f2pm")
        plpm = sb.tile([128, NSL, P], f32, tag="plpm")
        nc.sync.dma_start(
            l1pm[:], l1scr[:].rearrange("(j q g) p -> q j g p", g=2, q=128))
        nc.sync.dma_start(
            dotpm[:], dscr[:].rearrange("(j q g) p -> q j g p", g=2, q=128))
        nc.sync.dma_start(
            f2pm[:], f2scr[0:SP].rearrange("(j q g) -> q j g", g=2, q=128))
        nc.sync.dma_start(
            plpm[:], pl_d[:].rearrange("(s q) p -> q s p", q=128))

        # ---- phase D ----
        iot_i = sb.tile([128, P], i32, tag="ioti")
        nc.gpsimd.iota(iot_i[:], pattern=[[1, P]], base=0, channel_multiplier=0)
        iota_f = sb.tile([128, P], f32, tag="iotaf")
        nc.vector.tensor_copy(iota_f[:], iot_i[:])

        probs_pm = sb.tile([128, NS3, 2, P], f32, tag="probspm")
        lab_pm = sb.tile([128, NS3, 2], f32, tag="labpm")

        # sqrt stage (one ACT table load), Newton-refined to <=1 ulp
        fn_pm = sb.tile([128, NS3, 2], f32, tag="fnpm")
        pn_pm = sb.tile([128, NSL, P], f32, tag="pnpm")
        nc.scalar.activation(fn_pm[:], f2pm[:], Act.Sqrt)
        nc.scalar.activation(pn_pm[:], p2pm[:], Act.Sqrt)

        def newton_sqrt(s_ap, x_ap, tmp_tag, shape):
            for _ in range(2):
                q = sbd.tile(shape, f32, tag=tmp_tag)
                nc.vector.reciprocal(q[:], s_ap)
                nc.vector.tensor_tensor(out=q[:], in0=x_ap, in1=q[:], op=Alu.mult)
                nc.vector.tensor_tensor(out=q[:], in0=q[:], in1=s_ap, op=Alu.add)
                nc.vector.tensor_scalar(out=s_ap, in0=q[:], scalar1=0.5,
                                        scalar2=None, op0=Alu.mult)

        newton_sqrt(fn_pm[:], f2pm[:], "nsq_f", [128, NS3, 2])
        newton_sqrt(pn_pm[:], p2pm[:], "nsq_p", [128, NSL, P])
        nc.vector.tensor_scalar(out=fn_pm[:], in0=fn_pm[:], scalar1=1e-8,
                                scalar2=None, op0=Alu.max)
        nc.vector.tensor_scalar(out=pn_pm[:], in0=pn_pm[:], scalar1=1e-8,
                                scalar2=None, op0=Alu.max)

        for j in range(NS3):
            for g in range(2):
                dot = dotpm[:, j, g, :]
                l1v = l1pm[:, j, g, :]
                p2v = p2pm[:, j * 2 + g, :]
                f2v = f2pm[:, j, g:g + 1]
                fnv = fn_pm[:, j, g:g + 1]
                pnv = pn_pm[:, j * 2 + g, :]
                plv = plpm[:, j * 2 + g, :]

                w = sbd.tile([128, 3, P], f32, tag="w")
                t0 = sbd.tile([128, P], f32, tag="t0")
                t1 = sbd.tile([128, P], f32, tag="t1")

                # d_cos = 1 - dot/(fn*pn)
                nc.vector.tensor_scalar(out=t0[:], in0=pnv, scalar1=fnv,
                                        scalar2=None, op0=Alu.mult)
                nc.vector.reciprocal(t0[:], t0[:])
                nc.vector.tensor_tensor(out=t1[:], in0=dot, in1=t0[:],
                                        op=Alu.mult)
                nc.vector.tensor_scalar(out=w[:, 0, :], in0=t1[:],
                                        scalar1=-1.0, scalar2=1.0,
                                        op0=Alu.mult, op1=Alu.add)
                # d_l1 = l1/512
                nc.vector.tensor_scalar(out=w[:, 1, :], in0=l1v,
                                        scalar1=1.0 / 512, scalar2=None,
                                        op0=Alu.mult)
                # d_l2 = (f2 - 2 dot + p2)/512
                nc.vector.tensor_scalar(out=t0[:], in0=dot, scalar1=-2.0,
                                        scalar2=f2v, op0=Alu.mult, op1=Alu.add)
                nc.vector.tensor_tensor(out=t1[:], in0=t0[:], in1=p2v,
                                        op=Alu.add)
                nc.vector.tensor_scalar(out=w[:, 2, :], in0=t1[:],
                                        scalar1=1.0 / 512, scalar2=None,
                                        op0=Alu.mult)

                # logits = 1/(d + 1e-5)
                nc.vector.tensor_scalar(out=w[:], in0=w[:], scalar1=1e-5,
                                        scalar2=None, op0=Alu.add)
                nc.vector.reciprocal(w[:], w[:])

                acc = sbd.tile([128, P], f32, tag="acc")
                for r in range(3):
                    lr = w[:, r, :]
                    mx = sbd.tile([128, 1], f32, tag="mx")
                    nc.vector.tensor_reduce(out=mx[:], in_=lr, axis=X, op=Alu.max)
                    nc.vector.tensor_scalar(out=t0[:], in0=lr, scalar1=mx[:],
                                            scalar2=None, op0=Alu.subtract)
                    nc.scalar.activation(t0[:], t0[:], Act.Exp)
                    sm = sbd.tile([128, 1], f32, tag="sm")
                    nc.vector.tensor_reduce(out=sm[:], in_=t0[:], axis=X, op=Alu.add)
                    nc.vector.reciprocal(sm[:], sm[:])
                    nc.vector.tensor_scalar(out=t1[:], in0=t0[:], scalar1=sm[:],
                                            scalar2=None, op0=Alu.mult)
                    if r == 0:
                        nc.vector.tensor_copy(acc[:], t1[:])
                    else:
                        nc.vector.tensor_tensor(out=acc[:], in0=acc[:],
                                                in1=t1[:], op=Alu.add)
                pr = probs_pm[:, j, g, :]
                nc.vector.tensor_scalar(out=pr, in0=acc[:], scalar1=1.0 / 3.0,
                                        scalar2=None, op0=Alu.mult)

                # argmax (first max) + label gather
                mx = sbd.tile([128, 1], f32, tag="mx")
                nc.vector.tensor_reduce(out=mx[:], in_=pr, axis=X, op=Alu.max)
                msk = sbd.tile([128, P], f32, tag="msk")
                nc.vector.tensor_scalar(out=msk[:], in0=pr, scalar1=mx[:],
                                        scalar2=None, op0=Alu.is_ge)
                cand = sbd.tile([128, P], f32, tag="cand")
                nc.vector.tensor_scalar(out=t0[:], in0=msk[:], scalar1=-1e9,
                                        scalar2=1e9, op0=Alu.mult, op1=Alu.add)
                nc.vector.tensor_tensor(out=cand[:], in0=iota_f[:], in1=t0[:],
                                        op=Alu.add)
                idx = sbd.tile([128, 1], f32, tag="idx")
                nc.vector.tensor_reduce(out=idx[:], in_=cand[:], axis=X, op=Alu.min)
                nc.vector.tensor_scalar(out=msk[:], in0=iota_f[:], scalar1=idx[:],
                                        scalar2=None, op0=Alu.is_equal)
                nc.vector.tensor_tensor(out=cand[:], in0=msk[:], in1=plv,
                                        op=Alu.mult)
                nc.vector.tensor_reduce(out=lab_pm[:, j, g:g + 1], in_=cand[:],
                                        axis=X, op=Alu.add)

        # ---- phase E: outputs ----
        nc.sync.dma_start(
            probs_d[:].rearrange("(j g q) p -> q j g p", j=NS3, g=2, q=128),
            probs_pm[:])
        nc.sync.dma_start(
            lab_d[:].rearrange("(j g q) -> q j g", j=NS3, g=2, q=128),
            lab_pm[:])

    nc.compile()
    return nc


# --------------------------------------------------------------------------
# entry point
# --------------------------------------------------------------------------

def kernel(feats, protos, cls_ids, proto_labels):
    from concourse.bass_utils import run_bass_kernel_spmd

    feats = np.ascontiguousarray(np.asarray(feats, dtype=np.float32))
    protos = np.asarray(protos, dtype=np.float32)
    cls_np = np.asarray(cls_ids).astype(np.int64)
    pl_np = np.asarray(proto_labels)
    lab_dtype = pl_np.dtype

    core_tiles, NT = _plan(cls_np)
    CLSMAX = max(len({c for c, _ in ct}) for ct in core_tiles)
    protosT = protos.transpose(0, 2, 1).copy()        # [C, D, P]
    pl_f32 = pl_np.astype(np.float32)

    cores = [_core_inputs(ct, NT, CLSMAX, feats, protosT, pl_f32)
             for ct in core_tiles]

    key = (NT, CLSMAX)
    if key not in _BUILT:
        _BUILT[key] = _build(NT, CLSMAX)
    nc = _BUILT[key]

    in_maps = [
        {"ft": ci["ft"], "ptile": ci["ptile"], "pl": ci["pl"],
         "oneh": ci["oneh"], "pcomp": ci["pcomp"]}
        for ci in cores
    ]
    res = run_bass_kernel_spmd(nc, in_maps, core_ids=list(range(NCORES)))
    global LAST_RESULT
    LAST_RESULT = res

    probs = np.zeros((N, P), dtype=np.float32)
    labels = np.zeros((N,), dtype=np.float64)
    for ci, r in zip(cores, res.results):
        po, lo, sbx = r["probs_o"], r["lab_o"], ci["slot_box"]
        s_idx = np.nonzero(sbx >= 0)[0]
        b_idx = sbx[s_idx]
        rows = ((s_idx // 256) * 2 + (s_idx % 2)) * 128 + (s_idx % 256) // 2
        probs[b_idx] = po[rows]
        labels[b_idx] = lo[rows]
    return labels.astype(lab_dtype), probs
